# revision 64
# baseline (speedup 1.0000x reference)
"""Trainium2 Bass kernel for temporal-decay causal multi-head attention.

Problem: nn_MultiHeadAttention_9053791060340
  B=4, S=2048, DM=512, H=8, HD=64.
  out = softmax((Q K^T / sqrt(HD)) * exp(-rate*|t_i - t_j|) with causal mask) V,
  then out-projection.

Sharding: 8 cores = 4 batches x 2 head-groups (4 heads each). Each core
computes a partial out-projection [S, DM] for its head group; the host sums
the two partials per batch and adds the output bias.

Device algorithm (per core); matmul inputs in bf16 with fp32 PSUM
accumulation by default (set KERNEL_F32=1 for all-fp32, ~2.2x slower,
error ~1e-6 instead of ~3e-3 scale-relative):
  - scores are computed TRANSPOSED (S^T[k, q] = k . q), so softmax-free-axis
    tricks are unnecessary: we use a no-max softmax (scores here are bounded,
    |v| <~ 64, so exp never overflows in fp32), and the denominator comes for
    free from a ones-column appended to V (PV matmul directly accumulates
    ctx^T[hd, q] plus the row of denominators).
  - temporal decay factorizes on sorted days: exp(-r(t_i - t_k)) = a_i * b_k
    with a per-q-group reference t0 to keep the factors in fp32 range.
    a (and the 1/sqrt(HD) scale) is folded into q^T once; b is folded into a
    per-q-group scaled copy of k^T.
  - pairs far enough apart (rate*dist such that |s|*decay < 1e-4) have
    exp(s*decay) within 1e-4 of 1.0, so all k-chunks entirely below the
    cutoff collapse into a host-precomputed prefix: the V-prefix sum enters
    through the divide epilogue's scalar_tensor_tensor, the count through a
    scalar add on the gathered denominator rows.
  - causal masking needs work only in the diagonal 128x128 band of each
    q-group: an additive -1e30 tril tile is applied to S^T in PSUM before the
    exp. Blocks fully above the diagonal are skipped by restricting the
    streamed q-range.
  - the score/exp chain runs one k-chunk ahead of the PV chain in the PE
    FIFO so the PE is not head-of-line blocked on the ACT engine.
  - per-pair divide epilogue (no PE transpose dance): the two denominator
    rows are gathered onto one [2, QG] tile, reciprocal'd with the fast
    approximate DVE reciprocal (~22-bit), bounced through DRAM row-by-row
    (a stride-0 partition-broadcast DMA needs a DRAM source; GpSimd's
    partition_broadcast would thrash its microcode library against the
    tensor_tensor folds), multiplied into the ctx rows, and the head pair
    repacked onto one 128-partition tile so the out-projection runs K=128.
    Pair 0's whole chain hides under pair 1's score/exp stream; the
    out-projection of group qg is emitted mid-group qg+1.
  - final group: pair 1 skips the repack DMA (K=64 out-projection reads the
    halves directly), the PSUM->SBUF output casts run on the otherwise-idle
    ACT, and stores are split in half so the exposed drain stays short.
  - outputs are stored in bf16 (the host sums the two per-batch partials in
    f32); the added quantization is ~4e-4 of output scale.
"""

import os

import ml_dtypes
import numpy as np

import concourse.bass as bass
import concourse.tile as tile
from concourse import bacc
from concourse import mybir
from concourse.bass_utils import run_bass_kernel_spmd

F32 = mybir.dt.float32

B, S, DM, H = 4, 2048, 512, 8
HD = DM // H          # 64
NCORES = 8
NHG = 2               # head groups == cores per batch
NH = H // NHG         # heads per core
HGD = NH * HD         # 256 output dims per core
QG = 512              # q-group width
NQG = S // QG         # 4
KC = 128              # k chunk (partition dim of S^T)
NKC = S // KC         # 16
P = 128
NEG = -1.0e30

_cache: dict = {}


# --------------------------------------------------------------------------
# device program
# --------------------------------------------------------------------------

def _build_fast(kc_lo: tuple, wmax: int, with_bqk: bool, with_bv: bool,
                use_bf16: bool = True):
    """Build the SPMD Bass program.

    kc_lo[qg] = first near k-chunk per q-group (static across cores; chunks
    below it are covered by the prefix-sum rank-1 update). wmax = max
    near-window width in elements for the scaled-k tile. use_bf16 casts all
    matmul inputs to bf16 (fp32 PSUM accumulation): the PE streams 1 col/cyc
    for bf16 vs 2 for fp32, halving matmul time.

    Emission order is the scheduler priority order: projections and scaled-k
    prep run two q-groups ahead of the attention stream; each head pair's
    epilogue (reciprocal/divide/repack) is emitted right after its PV chain
    so it overlaps the next pair's score/exp stream; the out-projection of
    group qg is emitted in the middle of group qg+1 so only the last one is
    exposed at the end.
    """
    nc = bacc.Bacc()
    MDT = mybir.dt.bfloat16 if use_bf16 else F32

    xT_d = nc.declare_dram_parameter("xT", [DM, S], MDT, False)
    wq_d = nc.declare_dram_parameter("wqT", [DM, HGD], MDT, False)
    wk_d = nc.declare_dram_parameter("wkT", [DM, HGD], MDT, False)
    wv_d = nc.declare_dram_parameter("wvT", [DM, HGD], MDT, False)
    wo_d = nc.declare_dram_parameter("woT", [HGD, DM], MDT, False)
    av_d = nc.declare_dram_parameter("avec", [1, S], F32, False)
    bv_d = nc.declare_dram_parameter("bvec", [NQG, S], F32, False)
    pf_d = nc.declare_dram_parameter("prefv", [HD, NQG * NH], F32, False)
    ct_d = nc.declare_dram_parameter("cnt", [2, NQG * 2], F32, False)
    bm_d = nc.declare_dram_parameter("bandm", [P, P], F32, False)
    if with_bqk:
        bq_d = nc.declare_dram_parameter("bq", [P, 2], F32, False)
        bk_d = nc.declare_dram_parameter("bk", [P, 2], F32, False)
    if with_bv:
        bvb_d = nc.declare_dram_parameter("bvb", [1, HGD], F32, False)
    out_d = nc.declare_dram_parameter("outp", [S, DM], MDT, True)

    KO = DM // P  # 4 k-sub-chunks for DM-contraction
    VW = HD + 1   # 65: V columns plus ones column

    with tile.TileContext(nc) as tc:
        with (
            tc.tile_pool(name="const", bufs=1) as const,
            tc.tile_pool(name="ppool", bufs=2, space="PSUM") as ppool,
            tc.tile_pool(name="spool", bufs=2, space="PSUM") as spool,
            tc.tile_pool(name="cpool", bufs=2, space="PSUM") as cpool,
            tc.tile_pool(name="ptp", bufs=4) as ptp,
            tc.tile_pool(name="ktsp", bufs=3) as ktsp,
            tc.tile_pool(name="bvqp", bufs=3) as bvqp,
            tc.tile_pool(name="ctxp", bufs=6) as ctxp,
            tc.tile_pool(name="densp", bufs=3) as densp,
            tc.tile_pool(name="drp", bufs=2, space="DRAM") as drp,
        ):
            # ---- constant loads, ordered so group 0's dependencies land
            # first: wq/wk chunks, x slice 0, wv, then the rest ----
            wq_sb = const.tile([P, KO, HGD], MDT)
            wq_r = wq_d[:].rearrange("(ko p) m -> p ko m", p=P)
            wk_sb = const.tile([P, KO, HGD], MDT)
            wk_r = wk_d[:].rearrange("(ko p) m -> p ko m", p=P)
            for ki in range(KO):
                nc.sync.dma_start(wq_sb[:, ki, :], wq_r[:, ki, :])
                nc.sync.dma_start(wk_sb[:, ki, :], wk_r[:, ki, :])
            xT_sb = const.tile([P, KO, S], MDT)
            xT_r = xT_d[:].rearrange("(ko p) s -> p ko s", p=P)
            # slice 0 gates the very first projection matmuls: split it
            # across four queues so it lands ~4x sooner
            for ki in range(KO):
                nc.sync.dma_start(xT_sb[:, ki, 0:QG], xT_r[:, ki, 0:QG])
            wv_sb = const.tile([P, KO, HGD], MDT)
            nc.sync.dma_start(wv_sb, wv_d[:].rearrange("(ko p) m -> p ko m",
                                                       p=P))
            avec_full = const.tile([P, S], F32)
            nc.sync.dma_start(avec_full[:, 0:QG],
                              av_d[:][:, 0:QG].to_broadcast([P, QG]))
            bm_sb = const.tile([P, P], F32)
            nc.sync.dma_start(bm_sb, bm_d[:])
            for ns in range(1, 4):
                nc.sync.dma_start(xT_sb[:, :, ns * QG:(ns + 1) * QG],
                                  xT_r[:, :, ns * QG:(ns + 1) * QG])
                nc.sync.dma_start(avec_full[:, ns * QG:(ns + 1) * QG],
                                  av_d[:][:, ns * QG:(ns + 1) * QG]
                                  .to_broadcast([P, QG]))
            # head-pair on partitions: rows 0-63 = even head, 64-127 = odd
            wo_sb = const.tile([P, 2, DM], MDT)
            nc.sync.dma_start(wo_sb, wo_d[:].rearrange("(hp p) n -> p hp n",
                                                       p=P))
            # odd head of the last pair at partition base 0, for the final
            # group's un-repacked K=64 out-projection
            wo_o1 = const.tile([HD, DM], MDT)
            nc.sync.dma_start(wo_o1, wo_d[:][3 * HD:4 * HD, :])
            pft_sb = const.tile([HD, NQG * NH], F32)
            nc.sync.dma_start(pft_sb, pf_d[:])
            cnt2_sb = const.tile([2, NQG * 2], F32)
            nc.sync.dma_start(cnt2_sb, ct_d[:])
            if with_bqk:
                bq_sb = const.tile([P, 2], F32)
                nc.sync.dma_start(bq_sb, bq_d[:])
                bk_sb = const.tile([P, 2], F32)
                nc.sync.dma_start(bk_sb, bk_d[:])
            if with_bv:
                bv_full = const.tile([P, HGD], F32)
                nc.sync.dma_start(bv_full, bvb_d[:].to_broadcast([P, HGD]))

            # ---- projections (emitted per q-group, interleaved with the
            # ACT-bound attention stream so the PE stays dense) ----
            qT_f32 = const.tile([P, 2, S], F32)
            kT_sb = const.tile([P, 2, S], F32)
            qT_sb = const.tile([P, 2, S], MDT, name='qT_cast') if use_bf16 else qT_f32
            va_sb = const.tile([P, NKC, NH * VW], MDT)
            va_resh = va_sb.rearrange("p s (h c) -> p s h c", c=VW)
            nc.vector.memset(va_resh[:, :, :, HD], 1.0)

            def proj(ns):
                """q/k/v projections for sequence slice ns (one q-group)."""
                sl = slice(ns * QG, (ns + 1) * QG)
                for w_sb, t_sb, b_sb in (
                    (wq_sb, qT_f32, "q"),
                    (wk_sb, kT_sb, "k"),
                ):
                    for mc in range(2):
                        ps = ppool.tile([P, QG], F32, tag="pp")
                        for ki in range(KO):
                            nc.tensor.matmul(
                                ps,
                                lhsT=w_sb[:, ki, mc * P:(mc + 1) * P],
                                rhs=xT_sb[:, ki, sl],
                                start=(ki == 0),
                                stop=(ki == KO - 1),
                            )
                        # deprioritized elastic copies, on the DVE so the
                        # ACT engine stays clear for the exp stream
                        with tc.high_priority(offset=-600):
                            if with_bqk:
                                bias = (bq_sb if b_sb == "q"
                                        else bk_sb)[:, mc:mc + 1]
                                nc.scalar.activation(
                                    t_sb[:, mc, sl], ps,
                                    mybir.ActivationFunctionType.Identity,
                                    bias=bias,
                                )
                            else:
                                nc.scalar.copy(t_sb[:, mc, sl], ps)
                # fold a (and 1/sqrt(HD)) into q^T on the idle GPSIMD
                nc.gpsimd.tensor_tensor(
                    qT_sb[:, :, sl], qT_f32[:, :, sl],
                    avec_full[:, None, sl].to_broadcast([P, 2, QG]),
                    mybir.AluOpType.mult,
                )
                for sc in range(4 * ns, 4 * ns + 4):
                    ps = ppool.tile([P, HGD], F32, tag="pp")
                    for ki in range(KO):
                        nc.tensor.matmul(
                            ps,
                            lhsT=xT_sb[:, ki, sc * P:(sc + 1) * P],
                            rhs=wv_sb[:, ki, :],
                            start=(ki == 0),
                            stop=(ki == KO - 1),
                        )
                    with tc.high_priority(offset=-600):
                        # elastic: consumed by PVs a group later; keep them
                        # out of the band-adds' way in the DVE FIFO. One
                        # strided copy covers all four heads.
                        dst = va_resh[:, sc, :, 0:HD]
                        src = ps.rearrange("p (h c) -> p h c", c=HD)
                        if with_bv:
                            nc.vector.tensor_tensor(
                                dst, src,
                                bv_full.rearrange("p (h c) -> p h c", c=HD),
                                mybir.AluOpType.add,
                            )
                        else:
                            nc.vector.tensor_copy(dst, src)

            # ---- attention: score/exp/PV chains + inline pair epilogue --
            def prep(qg):
                """b-vector broadcast DMA + b-scaled k^T for group qg."""
                klo = kc_lo[qg] * KC
                khi = (qg + 1) * QG
                kw = khi - klo
                bvf = bvqp.tile([P, wmax], F32, tag="bvf")
                nc.sync.dma_start(
                    bvf[:, :kw],
                    bv_d[:][qg:qg + 1, klo:khi].to_broadcast([P, kw]),
                )
                kts = ktsp.tile([P, 2, wmax], MDT, tag="kts")
                nc.gpsimd.tensor_tensor(
                    kts[:, :, :kw], kT_sb[:, :, klo:khi],
                    bvf[:, None, :kw].to_broadcast([P, 2, kw]),
                    mybir.AluOpType.mult,
                )
                return kts

            def mk_score_exp(qg, kts):
                """build the score/exp emitter for one group (standalone so
                a group's first chunk can be emitted from inside the
                previous group's pipeline)."""
                klo = kc_lo[qg] * KC

                def score_exp(hp, kc):
                    """scores + band mask + exp for one k-chunk. Two heads
                    of one 128-row kT chunk run CONCURRENTLY on the PE via
                    row-tiling (array rows 0-63 / 64-127) and share one
                    2-bank score tile + one wide exp."""
                    diag = kc >= 4 * qg
                    q_off = max(0, KC * (kc - 4 * qg))
                    co = kc * KC - klo
                    sp2 = spool.tile([P, 2, QG], F32, tag="spsum")
                    for j in range(2):
                        pb = j * HD
                        nc.tensor.matmul(
                            sp2[:, j, q_off:],
                            lhsT=kts[pb:pb + HD, hp, co:co + KC],
                            rhs=qT_sb[pb:pb + HD, hp,
                                      qg * QG + q_off:(qg + 1) * QG],
                            start=True,
                            stop=True,
                        )
                    if diag:  # diagonal: mask both heads' bands
                        band = bass.AP(
                            tensor=sp2.tensor, offset=sp2.offset + q_off,
                            ap=[list(sp2.ap[0]), [QG, 2], [1, KC]],
                        )
                        nc.vector.tensor_tensor(
                            band, band, bm_sb[:, None, :].to_broadcast(
                                [P, 2, KC]),
                            mybir.AluOpType.add,
                        )
                    pt = ptp.tile([P, 2, QG], MDT, tag="pt")
                    nc.scalar.activation(
                        pt[:, :, q_off:], sp2[:, :, q_off:],
                        mybir.ActivationFunctionType.Exp,
                    )
                    return pt

                return score_exp

            def attn_group(qg, kts, outproj_cb=None, skip_repack_last=False,
                           prelude=None, prelude_cb=None):
                """both head pairs of one q-group as a single flat
                score/exp/PV software pipeline (the next pair's first
                score/exp is emitted before the previous pair's last PV, so
                the pair boundary costs no PE bubble), with the divide
                epilogue inline after each pair and outproj_cb (the previous
                group's out-projection) after pair 0's."""
                kcs = list(range(kc_lo[qg], 4 * (qg + 1)))
                items = [(hp, kc) for hp in range(2) for kc in kcs]
                cps = {}
                score_exp = mk_score_exp(qg, kts)

                def pv(hp, kc, pt):
                    q_off = max(0, KC * (kc - 4 * qg))
                    for j, h in enumerate((2 * hp, 2 * hp + 1)):
                        nc.tensor.matmul(
                            cps[hp][j][:, q_off:],
                            lhsT=va_sb[:, kc, h * VW:(h + 1) * VW],
                            rhs=pt[:, j, q_off:],
                            start=(kc == kcs[0]),
                            stop=(kc == kcs[-1]),
                        )

                def pair_epilogue(hp, skip_repack):
                    """fast approximate reciprocal of the denominators
                    (~22-bit, ample), DRAM-bounced partition broadcast
                    (stride-0 needs a DRAM source), divide+prefix-add, pair
                    repack for the K=128 out-projection. Pair 0's chain
                    hides under pair 1's score/exp stream."""
                    h0, h1 = 2 * hp, 2 * hp + 1
                    cxfs = []
                    dens2 = densp.tile([2, QG], F32, tag="dens2")
                    for j, h in enumerate((h0, h1)):
                        # undivided ctx AND den row to SBUF in one copy,
                        # freeing the accumulation bank (one on ACT, one on
                        # DVE so both banks free in parallel); den row ->
                        # dens2[j]. Deprioritized: must not queue ahead of
                        # the next pair's exps/band-adds.
                        cxf = ctxp.tile([VW, QG], F32, tag="cxf")
                        with tc.high_priority(offset=-600):
                            if j == 0:
                                nc.scalar.copy(cxf, cps[hp][j])
                            else:
                                nc.vector.tensor_copy(cxf, cps[hp][j])
                        nc.sync.dma_start(dens2[j:j + 1, :],
                                          cxf[HD:HD + 1, :])
                        cxfs.append(cxf)
                    nc.vector.tensor_scalar_add(dens2, dens2,
                                                cnt2_sb[:, 2 * qg + hp:
                                                        2 * qg + hp + 1])
                    rec = densp.tile([2, QG], F32, tag="rec")
                    nc.vector.reciprocal_approx_fast(rec, dens2)
                    recd = drp.tile([2, QG], F32, tag="recd")
                    for j in range(2):
                        nc.sync.dma_start(recd[j:j + 1, :], rec[j:j + 1, :])
                    cp2 = ctxp.tile([P, QG], MDT, tag="cpair")
                    csb_odd = None
                    for j, h in enumerate((h0, h1)):
                        bcs = densp.tile([HD, QG], F32, tag="bcs")
                        nc.sync.dma_start(
                            bcs, recd[j:j + 1, :].to_broadcast([HD, QG]))
                        # ctx = (near_ctx + distant_prefix) * (1/den)
                        dst = (cp2[0:HD, :] if j == 0
                               else ctxp.tile([HD, QG], MDT, tag="ctxsb"))
                        nc.vector.scalar_tensor_tensor(
                            dst, cxfs[j][:HD, :],
                            pft_sb[:HD, qg * NH + h:qg * NH + h + 1],
                            bcs,
                            mybir.AluOpType.add,
                            mybir.AluOpType.mult,
                        )
                        if j == 1:
                            if skip_repack:
                                csb_odd = dst
                            else:
                                nc.sync.dma_start(cp2[HD:P, :], dst)
                    return (cp2, csb_odd) if skip_repack else cp2

                results = [None, None]
                cps[0] = [cpool.tile([VW, QG], F32, tag="ctx", name="cpsA"),
                          cpool.tile([VW, QG], F32, tag="ctx", name="cpsB")]
                pts = [prelude if prelude is not None
                       else score_exp(*items[0])]
                for i, (hp, kc) in enumerate(items):
                    if i + 1 < len(items):
                        pts.append(score_exp(*items[i + 1]))
                    elif prelude_cb is not None:
                        # cross-group pipeline: the next group's first
                        # score/exp goes ahead of this group's last PV
                        prelude_cb()
                    if hp == 1 and 1 not in cps:
                        cps[1] = [cpool.tile([VW, QG], F32, tag="ctx",
                                             name="cpsC"),
                                  cpool.tile([VW, QG], F32, tag="ctx",
                                             name="cpsD")]
                    pv(hp, kc, pts[i])
                    if kc == kcs[-1]:
                        results[hp] = pair_epilogue(
                            hp, skip_repack_last and hp == 1)
                        if hp == 0 and outproj_cb is not None:
                            outproj_cb()
                return results

            def outproj(qg, pairs):
                """out-projection of one q-group from its two pair tiles."""
                last = qg == NQG - 1
                for ss in range(QG // P):
                    ops = ppool.tile([P, DM], F32, tag="pp")
                    nc.tensor.matmul(
                        ops,
                        lhsT=pairs[0][:, ss * P:(ss + 1) * P],
                        rhs=wo_sb[:, 0, :],
                        start=True,
                        stop=False,
                    )
                    if last:
                        # final group: pair 1 stays un-repacked (its repack
                        # DMA would sit on the exposed drain path); two K=64
                        # matmuls read the halves directly
                        cp2b, csb_odd = pairs[1]
                        nc.tensor.matmul(
                            ops, lhsT=cp2b[0:HD, ss * P:(ss + 1) * P],
                            rhs=wo_sb[0:HD, 1, :], start=False, stop=False,
                        )
                        nc.tensor.matmul(
                            ops, lhsT=csb_odd[:, ss * P:(ss + 1) * P],
                            rhs=wo_o1, start=False, stop=True,
                        )
                    else:
                        nc.tensor.matmul(
                            ops,
                            lhsT=pairs[1][:, ss * P:(ss + 1) * P],
                            rhs=wo_sb[:, 1, :],
                            start=False,
                            stop=True,
                        )
                    osb = ptp.tile([P, DM], MDT, tag="osb")
                    if last:
                        # keep the exposed drain off the DVE: cast on the
                        # otherwise-idle ACT, store in two half DMAs
                        nc.scalar.copy(osb, ops)
                        row = qg * QG + ss * P
                        nc.sync.dma_start(out_d[:][row:row + P, 0:DM // 2],
                                          osb[:, 0:DM // 2])
                        nc.sync.dma_start(out_d[:][row:row + P, DM // 2:],
                                          osb[:, DM // 2:])
                    else:
                        with tc.high_priority(offset=-600):
                            nc.vector.tensor_copy(osb, ops)
                        nc.sync.dma_start(
                            out_d[:][qg * QG + ss * P:
                                     qg * QG + (ss + 1) * P, :],
                            osb,
                        )

            # two-group lookahead: projections + scaled-k prefetch run well
            # ahead of the attention group that consumes them
            proj(0)
            ktss = [prep(0)]
            proj(1)
            ktss.append(prep(1))
            pending = None
            stash = {}
            for qg in range(NQG):
                cb = (None if pending is None
                      else (lambda g=qg - 1, p=pending: outproj(g, p)))
                pcb = None
                if qg + 1 < NQG:
                    def pcb(nxt=qg + 1):
                        se = mk_score_exp(nxt, ktss[nxt])
                        stash[nxt] = se(0, kc_lo[nxt])
                pairs = attn_group(qg, ktss[qg], outproj_cb=cb,
                                   skip_repack_last=(qg == NQG - 1),
                                   prelude=stash.pop(qg, None),
                                   prelude_cb=pcb)
                if qg + 2 < NQG:
                    proj(qg + 2)
                    ktss.append(prep(qg + 2))
                pending = pairs
            outproj(NQG - 1, pending)

    nc.finalize()
    return nc


# --------------------------------------------------------------------------
# host wrapper
# --------------------------------------------------------------------------

def _is_tril(mask: np.ndarray) -> bool:
    tril = np.tril(np.ones((S, S), dtype=mask.dtype))
    return all(np.array_equal(mask[b], tril) for b in range(mask.shape[0]))


def _prep_core_inputs(x, days, Wq, bq, Wk, bk, Wv, bv, Wo, rate,
                      use_bf16):
    """Per-core in_maps plus static loop bounds (shared across cores)."""
    t = days.astype(np.float64)  # [B, S]
    # distance beyond which |s| * decay < 1e-4, i.e. exp(s * decay) is
    # within 1e-4 of 1.0 (weight error ~1e-4 against a denominator >= the
    # window count), with a generous |s| <= 150 bound.
    d_cut = (np.log(150.0) + np.log(1.0e4)) / rate
    # static near-window bounds (min over batches so one program fits all)
    kc_lo = []
    for qg in range(NQG):
        lo = NKC
        for b in range(B):
            tq = t[b, qg * QG]
            c = 0
            while c < 4 * qg and t[b, c * KC + KC - 1] < tq - d_cut:
                c += 1
            lo = min(lo, c)
        kc_lo.append(lo)
    kc_lo = tuple(kc_lo)
    wmax = max((qg + 1) * QG - kc_lo[qg] * KC for qg in range(NQG))
    wmax = ((wmax + P - 1) // P) * P

    # per-batch decay factor vectors (f64 for exactness, then f32)
    scale = 1.0 / np.sqrt(HD)
    t0 = np.stack([(t[:, qg * QG] + t[:, qg * QG + QG - 1]) * 0.5
                   for qg in range(NQG)], axis=1)  # [B, NQG]
    avec = np.zeros((B, 1, S), np.float32)
    bvec = np.zeros((B, NQG, S), np.float32)
    for b in range(B):
        for qg in range(NQG):
            sl = slice(qg * QG, (qg + 1) * QG)
            avec[b, 0, sl] = (np.exp(-rate * (t[b, sl] - t0[b, qg])) * scale
                              ).astype(np.float32)
            hi = (qg + 1) * QG
            bvec[b, qg, :hi] = (np.exp(rate * (t[b, :hi] - t0[b, qg]))
                                ).astype(np.float32)
    assert np.all(np.isfinite(avec)) and np.all(np.isfinite(bvec)), \
        "decay factor overflow; q-group span too large for fast path"

    # band mask: keep (0.0) iff q_local >= k_local else -1e30
    kl = np.arange(P)[:, None]
    ql = np.arange(P)[None, :]
    bandm = np.where(ql >= kl, 0.0, NEG).astype(np.float32)

    with_bqk = bool(np.any(bq != 0) or np.any(bk != 0))
    with_bv = bool(np.any(bv != 0))

    in_maps = []
    for c in range(NCORES):
        b, hg = divmod(c, NHG)
        cols = slice(hg * HGD, (hg + 1) * HGD)
        # prefix V sums for the distant rank-1 update: [HD, NQG*NH] (hd-major)
        prefv = np.zeros((HD, NQG * NH), np.float32)
        cnt = np.zeros((2, NQG * 2), np.float32)
        for qg in range(NQG):
            n = kc_lo[qg] * KC
            cnt[:, 2 * qg:2 * qg + 2] = float(n)
            if n > 0:
                xs = x[b, :n].astype(np.float64).sum(axis=0)  # [DM]
                vs = xs @ Wv[cols, :].astype(np.float64).T \
                    + n * bv[cols].astype(np.float64)
                for h in range(NH):
                    prefv[:, qg * NH + h] = \
                        vs[h * HD:(h + 1) * HD].astype(np.float32)
        mdt = np.dtype(ml_dtypes.bfloat16) if use_bf16 else np.float32
        m = {
            "xT": np.ascontiguousarray(x[b].T).astype(mdt),
            "wqT": np.ascontiguousarray(Wq[cols, :].T).astype(mdt),
            "wkT": np.ascontiguousarray(Wk[cols, :].T).astype(mdt),
            "wvT": np.ascontiguousarray(Wv[cols, :].T).astype(mdt),
            "woT": np.ascontiguousarray(Wo[:, cols].T).astype(mdt),
            "avec": avec[b],
            "bvec": bvec[b],
            "prefv": prefv,
            "cnt": cnt,
            "bandm": bandm,
        }
        if with_bqk:
            m["bq"] = np.ascontiguousarray(
                bq[cols].reshape(2, P).T).astype(np.float32)
            m["bk"] = np.ascontiguousarray(
                bk[cols].reshape(2, P).T).astype(np.float32)
        if with_bv:
            m["bvb"] = bv[cols].reshape(1, HGD).astype(np.float32)
        in_maps.append(m)
    return in_maps, kc_lo, wmax, with_bqk, with_bv


def _reference_host(x, mask, days_offset, Wq, bq, Wk, bk, Wv, bv, Wo, bo,
                    decay_rate):
    """Emergency numpy fallback for inputs outside the fast path."""
    b, s, _ = x.shape
    out = np.empty((b, s, DM), np.float32)
    for bi in range(b):
        q = (x[bi] @ Wq.T + bq).reshape(s, H, HD).transpose(1, 0, 2)
        k = (x[bi] @ Wk.T + bk).reshape(s, H, HD).transpose(1, 0, 2)
        v = (x[bi] @ Wv.T + bv).reshape(s, H, HD).transpose(1, 0, 2)
        dist = np.abs(days_offset[bi][:, None] - days_offset[bi][None, :])
        decay = np.exp(-decay_rate * dist).astype(np.float32)
        ctx = np.empty((H, s, HD), np.float32)
        for h in range(H):
            sc = (q[h] @ k[h].T) / np.sqrt(HD) * decay
            sc = np.where(mask[bi] == 0, -np.inf, sc)
            sc = sc - sc.max(axis=-1, keepdims=True)
            e = np.exp(sc)
            ctx[h] = (e / e.sum(axis=-1, keepdims=True)) @ v[h]
        out[bi] = ctx.transpose(1, 0, 2).reshape(s, DM) @ Wo.T + bo
    return out


def kernel(x, mask, days_offset, Wq, bq, Wk, bk, Wv, bv, Wo, bo, decay_rate,
           _trace=False):
    x = np.asarray(x, np.float32)
    mask = np.asarray(mask)
    days = np.asarray(days_offset, np.float32)
    Wq, bq = np.asarray(Wq, np.float32), np.asarray(bq, np.float32)
    Wk, bk = np.asarray(Wk, np.float32), np.asarray(bk, np.float32)
    Wv, bv = np.asarray(Wv, np.float32), np.asarray(bv, np.float32)
    Wo, bo = np.asarray(Wo, np.float32), np.asarray(bo, np.float32)
    rate = float(np.asarray(decay_rate))

    sorted_ok = bool(np.all(np.diff(days, axis=-1) >= 0))
    if not (sorted_ok and _is_tril(mask)):
        return _reference_host(x, mask, days, Wq, bq, Wk, bk, Wv, bv, Wo, bo,
                               rate)

    use_bf16 = os.environ.get("KERNEL_F32", "") != "1"
    in_maps, kc_lo, wmax, with_bqk, with_bv = _prep_core_inputs(
        x, days, Wq, bq, Wk, bk, Wv, bv, Wo, rate, use_bf16)

    key = (kc_lo, wmax, with_bqk, with_bv, use_bf16)
    if key not in _cache:
        _cache[key] = _build_fast(kc_lo, wmax, with_bqk, with_bv, use_bf16)
    nc = _cache[key]

    res = run_bass_kernel_spmd(nc, in_maps, core_ids=list(range(NCORES)),
                               trace=_trace)
    out = np.empty((B, S, DM), np.float32)
    for b in range(B):
        out[b] = (res.results[2 * b]["outp"].astype(np.float32)
                  + res.results[2 * b + 1]["outp"].astype(np.float32) + bo)
    if _trace:
        return out, res
    return out


# revision 65
# speedup vs baseline: 1.1244x; 1.1244x over previous
"""Trainium2 Bass kernel for temporal-decay causal multi-head attention.

Problem: nn_MultiHeadAttention_9053791060340
  B=4, S=2048, DM=512, H=8, HD=64.
  out = softmax((Q K^T / sqrt(HD)) * exp(-rate*|t_i - t_j|) with causal mask) V,
  then out-projection.

Sharding: 8 cores = 4 batches x 2 head-groups (4 heads each). Each core
computes a partial out-projection [S, DM] for its head group; the host sums
the two partials per batch and adds the output bias.

Device algorithm (per core); matmul inputs in bf16 with fp32 PSUM
accumulation by default (set KERNEL_F32=1 for all-fp32, ~2.2x slower,
error ~1e-6 instead of ~3e-3 scale-relative):
  - scores are computed TRANSPOSED (S^T[k, q] = k . q), so softmax-free-axis
    tricks are unnecessary: we use a no-max softmax (scores here are bounded,
    |v| <~ 64, so exp never overflows in fp32), and the denominator comes for
    free from a ones-column appended to V (PV matmul directly accumulates
    ctx^T[hd, q] plus the row of denominators).
  - temporal decay factorizes on sorted days: exp(-r(t_i - t_k)) = a_i * b_k
    with a per-q-group reference t0 to keep the factors in fp32 range.
    a (and the 1/sqrt(HD) scale) is folded into q^T once; b is folded into a
    per-q-group scaled copy of k^T.
  - pairs far enough apart (rate*dist such that |s|*decay < 1e-4) have
    exp(s*decay) within 1e-4 of 1.0, so all k-chunks entirely below the
    cutoff collapse into a host-precomputed prefix: the V-prefix sum enters
    through the divide epilogue's scalar_tensor_tensor, the count through a
    scalar add on the gathered denominator rows.
  - causal masking needs work only in the diagonal 128x128 band of each
    q-group: an additive -1e30 tril tile is applied to S^T in PSUM before the
    exp. Blocks fully above the diagonal are skipped by restricting the
    streamed q-range.
  - the score/exp chain runs one k-chunk ahead of the PV chain in the PE
    FIFO so the PE is not head-of-line blocked on the ACT engine.
  - per-pair divide epilogue (no PE transpose dance): the two denominator
    rows are gathered onto one [2, QG] tile, reciprocal'd with the fast
    approximate DVE reciprocal (~22-bit), bounced through DRAM row-by-row
    (a stride-0 partition-broadcast DMA needs a DRAM source; GpSimd's
    partition_broadcast would thrash its microcode library against the
    tensor_tensor folds), multiplied into the ctx rows, and the head pair
    repacked onto one 128-partition tile so the out-projection runs K=128.
    Pair 0's whole chain hides under pair 1's score/exp stream; the
    out-projection of group qg is emitted mid-group qg+1.
  - final group: pair 1 skips the repack DMA (K=64 out-projection reads the
    halves directly), the PSUM->SBUF output casts run on the otherwise-idle
    ACT, and stores are split in half so the exposed drain stays short.
  - outputs are stored in bf16 (the host sums the two per-batch partials in
    f32); the added quantization is ~4e-4 of output scale.
"""

import os

import ml_dtypes
import numpy as np

import concourse.bass as bass
import concourse.tile as tile
from concourse import bacc
from concourse import mybir
from concourse.bass_utils import run_bass_kernel_spmd

F32 = mybir.dt.float32

B, S, DM, H = 4, 2048, 512, 8
HD = DM // H          # 64
NCORES = 8
NHG = 2               # head groups == cores per batch
NH = H // NHG         # heads per core
HGD = NH * HD         # 256 output dims per core
QG = 512              # q-group width
NQG = S // QG         # 4
KC = 128              # k chunk (partition dim of S^T)
NKC = S // KC         # 16
P = 128
NEG = -1.0e30

_cache: dict = {}


# --------------------------------------------------------------------------
# device program
# --------------------------------------------------------------------------

def _build_fast(kc_lo: tuple, wmax: int, with_bqk: bool, with_bv: bool,
                use_bf16: bool = True):
    """Build the SPMD Bass program.

    kc_lo[qg] = first near k-chunk per q-group (static across cores; chunks
    below it are covered by the prefix-sum rank-1 update). wmax = max
    near-window width in elements for the scaled-k tile. use_bf16 casts all
    matmul inputs to bf16 (fp32 PSUM accumulation): the PE streams 1 col/cyc
    for bf16 vs 2 for fp32, halving matmul time.

    Emission order is the scheduler priority order: projections and scaled-k
    prep run two q-groups ahead of the attention stream; each head pair's
    epilogue (reciprocal/divide/repack) is emitted right after its PV chain
    so it overlaps the next pair's score/exp stream; the out-projection of
    group qg is emitted in the middle of group qg+1 so only the last one is
    exposed at the end.
    """
    nc = bacc.Bacc()
    MDT = mybir.dt.bfloat16 if use_bf16 else F32

    xT_d = nc.declare_dram_parameter("xT", [DM, S], MDT, False)
    wq_d = nc.declare_dram_parameter("wqT", [DM, HGD], MDT, False)
    wk_d = nc.declare_dram_parameter("wkT", [DM, HGD], MDT, False)
    wv_d = nc.declare_dram_parameter("wvT", [DM, HGD], MDT, False)
    wo_d = nc.declare_dram_parameter("woT", [HGD, DM], MDT, False)
    av_d = nc.declare_dram_parameter("avec", [1, S], F32, False)
    bv_d = nc.declare_dram_parameter("bvec", [NQG, S], F32, False)
    pf_d = nc.declare_dram_parameter("prefv", [HD, NQG * NH], F32, False)
    ct_d = nc.declare_dram_parameter("cnt", [2, NQG * 2], F32, False)
    bm_d = nc.declare_dram_parameter("bandm", [P, P], F32, False)
    if with_bqk:
        bq_d = nc.declare_dram_parameter("bq", [P, 2], F32, False)
        bk_d = nc.declare_dram_parameter("bk", [P, 2], F32, False)
    if with_bv:
        bvb_d = nc.declare_dram_parameter("bvb", [1, HGD], F32, False)
    out_d = nc.declare_dram_parameter("outp", [S, DM], MDT, True)

    KO = DM // P  # 4 k-sub-chunks for DM-contraction
    VW = HD + 1   # 65: V columns plus ones column

    with tile.TileContext(nc) as tc:
        with (
            tc.tile_pool(name="const", bufs=1) as const,
            tc.tile_pool(name="ppool", bufs=2, space="PSUM") as ppool,
            tc.tile_pool(name="spool", bufs=2, space="PSUM") as spool,
            tc.tile_pool(name="cpool", bufs=2, space="PSUM") as cpool,
            tc.tile_pool(name="ptp", bufs=4) as ptp,
            tc.tile_pool(name="ktsp", bufs=3) as ktsp,
            tc.tile_pool(name="bvqp", bufs=3) as bvqp,
            tc.tile_pool(name="ctxp", bufs=6) as ctxp,
            tc.tile_pool(name="densp", bufs=3) as densp,
            tc.tile_pool(name="drp", bufs=2, space="DRAM") as drp,
        ):
            # ---- constant loads, ordered so group 0's dependencies land
            # first: wq/wk chunks, x slice 0, wv, then the rest ----
            wq_sb = const.tile([P, KO, HGD], MDT)
            wq_r = wq_d[:].rearrange("(ko p) m -> p ko m", p=P)
            wk_sb = const.tile([P, KO, HGD], MDT)
            wk_r = wk_d[:].rearrange("(ko p) m -> p ko m", p=P)
            for ki in range(KO):
                nc.sync.dma_start(wq_sb[:, ki, :], wq_r[:, ki, :])
                nc.sync.dma_start(wk_sb[:, ki, :], wk_r[:, ki, :])
            xT_sb = const.tile([P, KO, S], MDT)
            xT_r = xT_d[:].rearrange("(ko p) s -> p ko s", p=P)
            # slice 0 gates the very first projection matmuls: split it
            # across four queues so it lands ~4x sooner
            for ki in range(KO):
                nc.sync.dma_start(xT_sb[:, ki, 0:QG], xT_r[:, ki, 0:QG])
            wv_sb = const.tile([P, KO, HGD], MDT)
            nc.sync.dma_start(wv_sb, wv_d[:].rearrange("(ko p) m -> p ko m",
                                                       p=P))
            avec_full = const.tile([P, S], F32)
            nc.sync.dma_start(avec_full[:, 0:QG],
                              av_d[:][:, 0:QG].to_broadcast([P, QG]))
            bm_sb = const.tile([P, P], F32)
            nc.sync.dma_start(bm_sb, bm_d[:])
            for ns in range(1, 4):
                nc.sync.dma_start(xT_sb[:, :, ns * QG:(ns + 1) * QG],
                                  xT_r[:, :, ns * QG:(ns + 1) * QG])
                nc.sync.dma_start(avec_full[:, ns * QG:(ns + 1) * QG],
                                  av_d[:][:, ns * QG:(ns + 1) * QG]
                                  .to_broadcast([P, QG]))
            # head-pair on partitions: rows 0-63 = even head, 64-127 = odd
            wo_sb = const.tile([P, 2, DM], MDT)
            nc.sync.dma_start(wo_sb, wo_d[:].rearrange("(hp p) n -> p hp n",
                                                       p=P))
            # odd head of the last pair at partition base 0, for the final
            # group's un-repacked K=64 out-projection
            wo_o1 = const.tile([HD, DM], MDT)
            nc.sync.dma_start(wo_o1, wo_d[:][3 * HD:4 * HD, :])
            pft_sb = const.tile([HD, NQG * NH], F32)
            nc.sync.dma_start(pft_sb, pf_d[:])
            cnt2_sb = const.tile([2, NQG * 2], F32)
            nc.sync.dma_start(cnt2_sb, ct_d[:])
            if with_bqk:
                bq_sb = const.tile([P, 2], F32)
                nc.sync.dma_start(bq_sb, bq_d[:])
                bk_sb = const.tile([P, 2], F32)
                nc.sync.dma_start(bk_sb, bk_d[:])
            if with_bv:
                bv_full = const.tile([P, HGD], F32)
                nc.sync.dma_start(bv_full, bvb_d[:].to_broadcast([P, HGD]))

            # ---- projections (emitted per q-group, interleaved with the
            # ACT-bound attention stream so the PE stays dense) ----
            qT_f32 = const.tile([P, 2, S], F32)
            kT_sb = const.tile([P, 2, S], F32)
            qT_sb = const.tile([P, 2, S], MDT, name='qT_cast') if use_bf16 else qT_f32
            va_sb = const.tile([P, NKC, NH * VW], MDT)
            va_resh = va_sb.rearrange("p s (h c) -> p s h c", c=VW)
            nc.vector.memset(va_resh[:, :, :, HD], 1.0)

            def proj(ns):
                """q/k/v projections for sequence slice ns (one q-group)."""
                sl = slice(ns * QG, (ns + 1) * QG)
                for w_sb, t_sb, b_sb in (
                    (wq_sb, qT_f32, "q"),
                    (wk_sb, kT_sb, "k"),
                ):
                    for mc in range(2):
                        ps = ppool.tile([P, QG], F32, tag="pp")
                        for ki in range(KO):
                            nc.tensor.matmul(
                                ps,
                                lhsT=w_sb[:, ki, mc * P:(mc + 1) * P],
                                rhs=xT_sb[:, ki, sl],
                                start=(ki == 0),
                                stop=(ki == KO - 1),
                            )
                        # deprioritized elastic copies, on the DVE so the
                        # ACT engine stays clear for the exp stream
                        with tc.high_priority(offset=-600):
                            if with_bqk:
                                bias = (bq_sb if b_sb == "q"
                                        else bk_sb)[:, mc:mc + 1]
                                nc.scalar.activation(
                                    t_sb[:, mc, sl], ps,
                                    mybir.ActivationFunctionType.Identity,
                                    bias=bias,
                                )
                            else:
                                nc.scalar.copy(t_sb[:, mc, sl], ps)
                # fold a (and 1/sqrt(HD)) into q^T on the idle GPSIMD
                nc.gpsimd.tensor_tensor(
                    qT_sb[:, :, sl], qT_f32[:, :, sl],
                    avec_full[:, None, sl].to_broadcast([P, 2, QG]),
                    mybir.AluOpType.mult,
                )
                for sc in range(4 * ns, 4 * ns + 4):
                    ps = ppool.tile([P, HGD], F32, tag="pp")
                    for ki in range(KO):
                        nc.tensor.matmul(
                            ps,
                            lhsT=xT_sb[:, ki, sc * P:(sc + 1) * P],
                            rhs=wv_sb[:, ki, :],
                            start=(ki == 0),
                            stop=(ki == KO - 1),
                        )
                    with tc.high_priority(offset=-600):
                        # elastic: consumed by PVs a group later; keep them
                        # out of the band-adds' way in the DVE FIFO. One
                        # strided copy covers all four heads.
                        dst = va_resh[:, sc, :, 0:HD]
                        src = ps.rearrange("p (h c) -> p h c", c=HD)
                        if with_bv:
                            nc.vector.tensor_tensor(
                                dst, src,
                                bv_full.rearrange("p (h c) -> p h c", c=HD),
                                mybir.AluOpType.add,
                            )
                        else:
                            nc.vector.tensor_copy(dst, src)

            # ---- attention: score/exp/PV chains + inline pair epilogue --
            def prep(qg):
                """b-vector broadcast DMA + b-scaled k^T for group qg."""
                klo = kc_lo[qg] * KC
                khi = (qg + 1) * QG
                kw = khi - klo
                bvf = bvqp.tile([P, wmax], F32, tag="bvf")
                nc.sync.dma_start(
                    bvf[:, :kw],
                    bv_d[:][qg:qg + 1, klo:khi].to_broadcast([P, kw]),
                )
                kts = ktsp.tile([P, 2, wmax], MDT, tag="kts")
                nc.gpsimd.tensor_tensor(
                    kts[:, :, :kw], kT_sb[:, :, klo:khi],
                    bvf[:, None, :kw].to_broadcast([P, 2, kw]),
                    mybir.AluOpType.mult,
                )
                return kts

            def mk_score_exp(qg, kts):
                """build the score/exp emitter for one group (standalone so
                a group's first chunk can be emitted from inside the
                previous group's pipeline)."""
                klo = kc_lo[qg] * KC

                def score_exp(hp, kc):
                    """scores + band mask + exp for one k-chunk. Two heads
                    of one 128-row kT chunk run CONCURRENTLY on the PE via
                    row-tiling (array rows 0-63 / 64-127) and share one
                    2-bank score tile + one wide exp."""
                    diag = kc >= 4 * qg
                    q_off = max(0, KC * (kc - 4 * qg))
                    co = kc * KC - klo
                    sp2 = spool.tile([P, 2, QG], F32, tag="spsum")
                    for j in range(2):
                        pb = j * HD
                        nc.tensor.matmul(
                            sp2[:, j, q_off:],
                            lhsT=kts[pb:pb + HD, hp, co:co + KC],
                            rhs=qT_sb[pb:pb + HD, hp,
                                      qg * QG + q_off:(qg + 1) * QG],
                            start=True,
                            stop=True,
                        )
                    if diag:  # diagonal: mask both heads' bands
                        band = bass.AP(
                            tensor=sp2.tensor, offset=sp2.offset + q_off,
                            ap=[list(sp2.ap[0]), [QG, 2], [1, KC]],
                        )
                        nc.vector.tensor_tensor(
                            band, band, bm_sb[:, None, :].to_broadcast(
                                [P, 2, KC]),
                            mybir.AluOpType.add,
                        )
                    pt = ptp.tile([P, 2, QG], MDT, tag="pt")
                    nc.scalar.activation(
                        pt[:, :, q_off:], sp2[:, :, q_off:],
                        mybir.ActivationFunctionType.Exp,
                    )
                    return pt

                return score_exp

            def attn_group(qg, kts, outproj_cb=None, skip_repack_last=False,
                           prelude=None, prelude_cb=None):
                """both head pairs of one q-group as a single flat
                score/exp/PV software pipeline (the next pair's first
                score/exp is emitted before the previous pair's last PV, so
                the pair boundary costs no PE bubble), with the divide
                epilogue inline after each pair and outproj_cb (the previous
                group's out-projection) after pair 0's."""
                kcs = list(range(kc_lo[qg], 4 * (qg + 1)))
                items = [(hp, kc) for hp in range(2) for kc in kcs]
                cps = {}
                score_exp = mk_score_exp(qg, kts)

                def pv(hp, kc, pt):
                    q_off = max(0, KC * (kc - 4 * qg))
                    for j, h in enumerate((2 * hp, 2 * hp + 1)):
                        nc.tensor.matmul(
                            cps[hp][j][:, q_off:],
                            lhsT=va_sb[:, kc, h * VW:(h + 1) * VW],
                            rhs=pt[:, j, q_off:],
                            start=(kc == kcs[0]),
                            stop=(kc == kcs[-1]),
                        )

                def pair_epilogue(hp, skip_repack):
                    """fast approximate reciprocal of the denominators
                    (~22-bit, ample), DRAM-bounced partition broadcast
                    (stride-0 needs a DRAM source), divide+prefix-add, pair
                    repack for the K=128 out-projection. Pair 0's chain
                    hides under pair 1's score/exp stream."""
                    h0, h1 = 2 * hp, 2 * hp + 1
                    cxfs = []
                    dens2 = densp.tile([2, QG], F32, tag="dens2")
                    for j, h in enumerate((h0, h1)):
                        # undivided ctx AND den row to SBUF in one copy,
                        # freeing the accumulation bank (one on ACT, one on
                        # DVE so both banks free in parallel); den row ->
                        # dens2[j]. Deprioritized: must not queue ahead of
                        # the next pair's exps/band-adds.
                        cxf = ctxp.tile([VW, QG], F32, tag="cxf")
                        with tc.high_priority(offset=-600):
                            if j == 0:
                                nc.scalar.copy(cxf, cps[hp][j])
                            else:
                                nc.vector.tensor_copy(cxf, cps[hp][j])
                        nc.sync.dma_start(dens2[j:j + 1, :],
                                          cxf[HD:HD + 1, :])
                        cxfs.append(cxf)
                    nc.vector.tensor_scalar_add(dens2, dens2,
                                                cnt2_sb[:, 2 * qg + hp:
                                                        2 * qg + hp + 1])
                    rec = densp.tile([2, QG], F32, tag="rec")
                    nc.vector.reciprocal_approx_fast(rec, dens2)
                    recd = drp.tile([2, QG], F32, tag="recd")
                    for j in range(2):
                        nc.sync.dma_start(recd[j:j + 1, :], rec[j:j + 1, :])
                    cp2 = ctxp.tile([P, QG], MDT, tag="cpair")
                    csb_odd = None
                    for j, h in enumerate((h0, h1)):
                        bcs = densp.tile([HD, QG], F32, tag="bcs")
                        nc.sync.dma_start(
                            bcs, recd[j:j + 1, :].to_broadcast([HD, QG]))
                        # ctx = (near_ctx + distant_prefix) * (1/den)
                        dst = (cp2[0:HD, :] if j == 0
                               else ctxp.tile([HD, QG], MDT, tag="ctxsb"))
                        nc.vector.scalar_tensor_tensor(
                            dst, cxfs[j][:HD, :],
                            pft_sb[:HD, qg * NH + h:qg * NH + h + 1],
                            bcs,
                            mybir.AluOpType.add,
                            mybir.AluOpType.mult,
                        )
                        if j == 1:
                            if skip_repack:
                                csb_odd = dst
                            else:
                                nc.sync.dma_start(cp2[HD:P, :], dst)
                    return (cp2, csb_odd) if skip_repack else cp2

                results = [None, None]
                cps[0] = [cpool.tile([VW, QG], F32, tag="ctx", name="cpsA"),
                          cpool.tile([VW, QG], F32, tag="ctx", name="cpsB")]
                pts = [prelude if prelude is not None
                       else score_exp(*items[0])]
                for i, (hp, kc) in enumerate(items):
                    if i + 1 < len(items):
                        pts.append(score_exp(*items[i + 1]))
                    elif prelude_cb is not None:
                        # cross-group pipeline: the next group's first
                        # score/exp goes ahead of this group's last PV
                        prelude_cb()
                    if hp == 1 and 1 not in cps:
                        cps[1] = [cpool.tile([VW, QG], F32, tag="ctx",
                                             name="cpsC"),
                                  cpool.tile([VW, QG], F32, tag="ctx",
                                             name="cpsD")]
                    pv(hp, kc, pts[i])
                    if kc == kcs[-1]:
                        results[hp] = pair_epilogue(
                            hp, skip_repack_last and hp == 1)
                        if hp == 0 and outproj_cb is not None:
                            outproj_cb()
                return results

            def outproj(qg, pairs):
                """out-projection of one q-group from its two pair tiles."""
                last = qg == NQG - 1
                for ss in range(QG // P):
                    ops = ppool.tile([P, DM], F32, tag="pp")
                    nc.tensor.matmul(
                        ops,
                        lhsT=pairs[0][:, ss * P:(ss + 1) * P],
                        rhs=wo_sb[:, 0, :],
                        start=True,
                        stop=False,
                    )
                    if last:
                        # final group: pair 1 stays un-repacked (its repack
                        # DMA would sit on the exposed drain path); two K=64
                        # matmuls read the halves directly
                        cp2b, csb_odd = pairs[1]
                        nc.tensor.matmul(
                            ops, lhsT=cp2b[0:HD, ss * P:(ss + 1) * P],
                            rhs=wo_sb[0:HD, 1, :], start=False, stop=False,
                        )
                        nc.tensor.matmul(
                            ops, lhsT=csb_odd[:, ss * P:(ss + 1) * P],
                            rhs=wo_o1, start=False, stop=True,
                        )
                    else:
                        nc.tensor.matmul(
                            ops,
                            lhsT=pairs[1][:, ss * P:(ss + 1) * P],
                            rhs=wo_sb[:, 1, :],
                            start=False,
                            stop=True,
                        )
                    osb = ptp.tile([P, DM], MDT, tag="osb")
                    if last:
                        # keep the exposed drain off the DVE: cast on the
                        # otherwise-idle ACT, store in two half DMAs
                        nc.scalar.copy(osb, ops)
                        row = qg * QG + ss * P
                        nc.sync.dma_start(out_d[:][row:row + P, 0:DM // 2],
                                          osb[:, 0:DM // 2])
                        nc.sync.dma_start(out_d[:][row:row + P, DM // 2:],
                                          osb[:, DM // 2:])
                    else:
                        with tc.high_priority(offset=-600):
                            nc.vector.tensor_copy(osb, ops)
                        nc.sync.dma_start(
                            out_d[:][qg * QG + ss * P:
                                     qg * QG + (ss + 1) * P, :],
                            osb,
                        )

            # two-group lookahead: projections + scaled-k prefetch run well
            # ahead of the attention group that consumes them
            proj(0)
            ktss = [prep(0)]
            proj(1)
            ktss.append(prep(1))
            pending = None
            for qg in range(NQG):
                cb = (None if pending is None
                      else (lambda g=qg - 1, p=pending: outproj(g, p)))
                pairs = attn_group(qg, ktss[qg], outproj_cb=cb,
                                   skip_repack_last=(qg == NQG - 1))
                if qg + 2 < NQG:
                    proj(qg + 2)
                    ktss.append(prep(qg + 2))
                pending = pairs
            outproj(NQG - 1, pending)

    nc.finalize()
    return nc


# --------------------------------------------------------------------------
# host wrapper
# --------------------------------------------------------------------------

def _is_tril(mask: np.ndarray) -> bool:
    tril = np.tril(np.ones((S, S), dtype=mask.dtype))
    return all(np.array_equal(mask[b], tril) for b in range(mask.shape[0]))


def _prep_core_inputs(x, days, Wq, bq, Wk, bk, Wv, bv, Wo, rate,
                      use_bf16):
    """Per-core in_maps plus static loop bounds (shared across cores)."""
    t = days.astype(np.float64)  # [B, S]
    # distance beyond which |s| * decay < 1e-4, i.e. exp(s * decay) is
    # within 1e-4 of 1.0 (weight error ~1e-4 against a denominator >= the
    # window count), with a generous |s| <= 150 bound.
    d_cut = (np.log(150.0) + np.log(1.0e4)) / rate
    # static near-window bounds (min over batches so one program fits all)
    kc_lo = []
    for qg in range(NQG):
        lo = NKC
        for b in range(B):
            tq = t[b, qg * QG]
            c = 0
            while c < 4 * qg and t[b, c * KC + KC - 1] < tq - d_cut:
                c += 1
            lo = min(lo, c)
        kc_lo.append(lo)
    kc_lo = tuple(kc_lo)
    wmax = max((qg + 1) * QG - kc_lo[qg] * KC for qg in range(NQG))
    wmax = ((wmax + P - 1) // P) * P

    # per-batch decay factor vectors (f64 for exactness, then f32)
    scale = 1.0 / np.sqrt(HD)
    t0 = np.stack([(t[:, qg * QG] + t[:, qg * QG + QG - 1]) * 0.5
                   for qg in range(NQG)], axis=1)  # [B, NQG]
    avec = np.zeros((B, 1, S), np.float32)
    bvec = np.zeros((B, NQG, S), np.float32)
    for b in range(B):
        for qg in range(NQG):
            sl = slice(qg * QG, (qg + 1) * QG)
            avec[b, 0, sl] = (np.exp(-rate * (t[b, sl] - t0[b, qg])) * scale
                              ).astype(np.float32)
            hi = (qg + 1) * QG
            bvec[b, qg, :hi] = (np.exp(rate * (t[b, :hi] - t0[b, qg]))
                                ).astype(np.float32)
    assert np.all(np.isfinite(avec)) and np.all(np.isfinite(bvec)), \
        "decay factor overflow; q-group span too large for fast path"

    # band mask: keep (0.0) iff q_local >= k_local else -1e30
    kl = np.arange(P)[:, None]
    ql = np.arange(P)[None, :]
    bandm = np.where(ql >= kl, 0.0, NEG).astype(np.float32)

    with_bqk = bool(np.any(bq != 0) or np.any(bk != 0))
    with_bv = bool(np.any(bv != 0))

    in_maps = []
    for c in range(NCORES):
        b, hg = divmod(c, NHG)
        cols = slice(hg * HGD, (hg + 1) * HGD)
        # prefix V sums for the distant rank-1 update: [HD, NQG*NH] (hd-major)
        prefv = np.zeros((HD, NQG * NH), np.float32)
        cnt = np.zeros((2, NQG * 2), np.float32)
        for qg in range(NQG):
            n = kc_lo[qg] * KC
            cnt[:, 2 * qg:2 * qg + 2] = float(n)
            if n > 0:
                xs = x[b, :n].astype(np.float64).sum(axis=0)  # [DM]
                vs = xs @ Wv[cols, :].astype(np.float64).T \
                    + n * bv[cols].astype(np.float64)
                for h in range(NH):
                    prefv[:, qg * NH + h] = \
                        vs[h * HD:(h + 1) * HD].astype(np.float32)
        mdt = np.dtype(ml_dtypes.bfloat16) if use_bf16 else np.float32
        m = {
            "xT": np.ascontiguousarray(x[b].T).astype(mdt),
            "wqT": np.ascontiguousarray(Wq[cols, :].T).astype(mdt),
            "wkT": np.ascontiguousarray(Wk[cols, :].T).astype(mdt),
            "wvT": np.ascontiguousarray(Wv[cols, :].T).astype(mdt),
            "woT": np.ascontiguousarray(Wo[:, cols].T).astype(mdt),
            "avec": avec[b],
            "bvec": bvec[b],
            "prefv": prefv,
            "cnt": cnt,
            "bandm": bandm,
        }
        if with_bqk:
            m["bq"] = np.ascontiguousarray(
                bq[cols].reshape(2, P).T).astype(np.float32)
            m["bk"] = np.ascontiguousarray(
                bk[cols].reshape(2, P).T).astype(np.float32)
        if with_bv:
            m["bvb"] = bv[cols].reshape(1, HGD).astype(np.float32)
        in_maps.append(m)
    return in_maps, kc_lo, wmax, with_bqk, with_bv


def _reference_host(x, mask, days_offset, Wq, bq, Wk, bk, Wv, bv, Wo, bo,
                    decay_rate):
    """Emergency numpy fallback for inputs outside the fast path."""
    b, s, _ = x.shape
    out = np.empty((b, s, DM), np.float32)
    for bi in range(b):
        q = (x[bi] @ Wq.T + bq).reshape(s, H, HD).transpose(1, 0, 2)
        k = (x[bi] @ Wk.T + bk).reshape(s, H, HD).transpose(1, 0, 2)
        v = (x[bi] @ Wv.T + bv).reshape(s, H, HD).transpose(1, 0, 2)
        dist = np.abs(days_offset[bi][:, None] - days_offset[bi][None, :])
        decay = np.exp(-decay_rate * dist).astype(np.float32)
        ctx = np.empty((H, s, HD), np.float32)
        for h in range(H):
            sc = (q[h] @ k[h].T) / np.sqrt(HD) * decay
            sc = np.where(mask[bi] == 0, -np.inf, sc)
            sc = sc - sc.max(axis=-1, keepdims=True)
            e = np.exp(sc)
            ctx[h] = (e / e.sum(axis=-1, keepdims=True)) @ v[h]
        out[bi] = ctx.transpose(1, 0, 2).reshape(s, DM) @ Wo.T + bo
    return out


def kernel(x, mask, days_offset, Wq, bq, Wk, bk, Wv, bv, Wo, bo, decay_rate,
           _trace=False):
    x = np.asarray(x, np.float32)
    mask = np.asarray(mask)
    days = np.asarray(days_offset, np.float32)
    Wq, bq = np.asarray(Wq, np.float32), np.asarray(bq, np.float32)
    Wk, bk = np.asarray(Wk, np.float32), np.asarray(bk, np.float32)
    Wv, bv = np.asarray(Wv, np.float32), np.asarray(bv, np.float32)
    Wo, bo = np.asarray(Wo, np.float32), np.asarray(bo, np.float32)
    rate = float(np.asarray(decay_rate))

    sorted_ok = bool(np.all(np.diff(days, axis=-1) >= 0))
    if not (sorted_ok and _is_tril(mask)):
        return _reference_host(x, mask, days, Wq, bq, Wk, bk, Wv, bv, Wo, bo,
                               rate)

    use_bf16 = os.environ.get("KERNEL_F32", "") != "1"
    in_maps, kc_lo, wmax, with_bqk, with_bv = _prep_core_inputs(
        x, days, Wq, bq, Wk, bk, Wv, bv, Wo, rate, use_bf16)

    key = (kc_lo, wmax, with_bqk, with_bv, use_bf16)
    if key not in _cache:
        _cache[key] = _build_fast(kc_lo, wmax, with_bqk, with_bv, use_bf16)
    nc = _cache[key]

    res = run_bass_kernel_spmd(nc, in_maps, core_ids=list(range(NCORES)),
                               trace=_trace)
    out = np.empty((B, S, DM), np.float32)
    for b in range(B):
        out[b] = (res.results[2 * b]["outp"].astype(np.float32)
                  + res.results[2 * b + 1]["outp"].astype(np.float32) + bo)
    if _trace:
        return out, res
    return out
